# revision 2
# baseline (speedup 1.0000x reference)
"""GCN node classification on 8 Trainium2 NeuronCores (Bass/Tile).

Strategy (dst-sharded graph parallel):
  - Nodes padded to 100352 = 8 * 12544; core c owns dst nodes
    [c*12544, (c+1)*12544)  (98 tiles of 128).
  - Per layer: each core computes xw = g_own @ W on PE, AllGather makes the
    full [100352, F] feature table resident on every core's HBM.
  - Edges (incl. self-loops with coef 2*dinv^2, sorted by (src-window,
    dst-tile)) are processed in 128-edge chunks: a dma_gather pulls the
    source rows (int16 idx relative to one of four 25088-row windows,
    spread over 4 SWDGE queues), DVE builds M[e,d] = coef[e] * (d ==
    dst_local[e]) via one fused tensor_scalar, and PE computes
    psum += M^T @ Y, accumulating the weighted segment sum. Runs flush
    psum into an SBUF-resident per-core aggregate.
  - Epilogue per tile: +bias, +residual, erf-GELU, PE transpose, next
    layer's matmul, DMA into the next collective's input buffer.
"""
import sys
import time

sys.path.insert(0, "/opt/trn_rl_repo")

import numpy as np

import concourse.bass as bass  # noqa: E402
import concourse.tile as tile  # noqa: E402
from concourse import bacc, mybir  # noqa: E402
from concourse.bass_utils import run_bass_kernel_spmd  # noqa: E402

NCORES = 8
F = 128          # feature width (all layers padded to 128)
TILES = 98       # dst tiles per core
OWN = TILES * 128            # 12544 nodes per core
NT = NCORES * OWN            # 100352 padded nodes
NWIN = 4
WIN = 25088                  # src window (int16-addressable, < 32768)
GGROUP = 16                  # chunks per dma_gather call (2048 idxs)
C_OUT = 40


# --------------------------------------------------------------------------
# host-side preprocessing
# --------------------------------------------------------------------------

def preprocess(x, edge_index, n_real):
    """Shard + schedule. Returns (per_core_data, schedule).

    schedule: list over q of list of (tile, K) runs; shared by all cores.
    per_core: dict with idx16, dstl, coef arrays laid out per the schedule.
    """
    src = np.asarray(edge_index[0], dtype=np.int64)
    dst = np.asarray(edge_index[1], dtype=np.int64)
    E = src.shape[0]

    deg = np.bincount(dst, minlength=NT).astype(np.float32) + 2.0
    dinv = 1.0 / np.sqrt(deg)

    # self loops for real nodes only
    sl = np.arange(n_real, dtype=np.int64)
    src_a = np.concatenate([src, sl])
    dst_a = np.concatenate([dst, sl])
    coef_a = np.concatenate([
        dinv[src] * dinv[dst],
        2.0 * dinv[sl] * dinv[sl],
    ]).astype(np.float32)

    core = dst_a // OWN
    dstl_a = dst_a - core * OWN
    t_a = dstl_a >> 7
    dloc_a = (dstl_a & 127).astype(np.float32)
    q_a = np.minimum(src_a // WIN, NWIN - 1)
    # NOTE: src in [3*25088, 100352) -> q=3, idx = src - 75264 < 25088 ok
    idxrel_a = (src_a - q_a * WIN).astype(np.int64)
    assert idxrel_a.max() < 32768

    # counts[c, t, q]
    counts = np.zeros((NCORES, TILES, NWIN), dtype=np.int64)
    np.add.at(counts, (core, t_a, q_a), 1)
    K = np.ceil(counts.max(axis=0) / 128).astype(np.int64)  # [TILES, NWIN]
    K = np.maximum(K, 0)

    schedule = []
    for q in range(NWIN):
        runs = [(t, int(K[t, q])) for t in range(TILES) if K[t, q] > 0]
        schedule.append(runs)
    CH = int(K.sum())

    # per-core edge placement
    per_core = []
    # slot base of each (q) stream and (t,q) run in chunk units
    run_chunk_base = {}
    cbase = 0
    for q in range(NWIN):
        for t, k in schedule[q]:
            run_chunk_base[(q, t)] = cbase
            cbase += k
    assert cbase == CH

    order = np.lexsort((idxrel_a, t_a, q_a, core))
    src_s = idxrel_a[order]
    core_s = core[order]
    t_s = t_a[order]
    q_s = q_a[order]
    dloc_s = dloc_a[order]
    coef_s = coef_a[order]

    for c in range(NCORES):
        sel = core_s == c
        idx16 = np.zeros(CH * 128, dtype=np.int16)
        dstl = np.zeros(CH * 128, dtype=np.float32)
        coefv = np.zeros(CH * 128, dtype=np.float32)
        # bucket offsets for this core
        csel_t = t_s[sel]
        csel_q = q_s[sel]
        csel_i = src_s[sel]
        csel_d = dloc_s[sel]
        csel_c = coef_s[sel]
        # edges within (q,t) are contiguous (sorted); find boundaries
        key = csel_q * TILES + csel_t
        bounds = np.flatnonzero(np.r_[True, key[1:] != key[:-1], True])
        for bi in range(len(bounds) - 1):
            lo, hi = bounds[bi], bounds[bi + 1]
            q0, t0 = int(csel_q[lo]), int(csel_t[lo])
            base = run_chunk_base[(q0, t0)] * 128
            n = hi - lo
            assert n <= (run_chunk_base.get((q0, t0), 0) * 0 + 1) * 10**9
            idx16[base:base + n] = csel_i[lo:hi]
            dstl[base:base + n] = csel_d[lo:hi]
            coefv[base:base + n] = csel_c[lo:hi]
        per_core.append({"idx16": idx16, "dstl": dstl, "coef": coefv})

    return per_core, schedule, CH


def _wrap_idx(flat_idx, call_slices):
    """Lay out int16 idxs per gather call (16-partition wrap, x8 replicate)."""
    CHtot = flat_idx.shape[0] // 128
    out = np.zeros((128, CHtot * 8), dtype=np.int16)
    for (c0, c1) in call_slices:
        blk = flat_idx[c0 * 128:c1 * 128]
        w = blk.reshape(-1, 16).T  # [16, n*8]
        out[:, c0 * 8:c1 * 8] = np.tile(w, (8, 1))
    return out


def _call_plan(schedule):
    """Split each q-stream into gather calls of <= GGROUP chunks.

    Returns list of (q, chunk_lo, chunk_hi) in global chunk coordinates.
    """
    calls = []
    cbase = 0
    for q in range(NWIN):
        qlen = sum(k for _, k in schedule[q])
        pos = 0
        while pos < qlen:
            n = min(GGROUP, qlen - pos)
            calls.append((q, cbase + pos, cbase + pos + n))
            pos += n
        cbase += qlen
    return calls


# --------------------------------------------------------------------------
# bass program
# --------------------------------------------------------------------------

def build(schedule, CH):
    calls = _call_plan(schedule)
    nc = bacc.Bacc("TRN2", target_bir_lowering=False, debug=False,
                   num_devices=NCORES, num_swdge_queues=4)

    xT_in = nc.dram_tensor("xT", [128, OWN], mybir.dt.float32, kind="ExternalInput")
    idx16_in = nc.dram_tensor("idx16", [128, CH * 8], mybir.dt.int16, kind="ExternalInput")
    dstl_in = nc.dram_tensor("dstl", [128, CH], mybir.dt.float32, kind="ExternalInput")
    coef_in = nc.dram_tensor("coef", [128, CH], mybir.dt.float32, kind="ExternalInput")
    w_in = [nc.dram_tensor(f"W{l}", [128, 128], mybir.dt.float32, kind="ExternalInput")
            for l in range(4)]
    b_in = [nc.dram_tensor(f"b{l}", [128, 128], mybir.dt.float32, kind="ExternalInput")
            for l in range(4)]
    iota_in = nc.dram_tensor("iota", [128, 128], mybir.dt.float32, kind="ExternalInput")
    ident_in = nc.dram_tensor("ident", [128, 128], mybir.dt.float32, kind="ExternalInput")
    out_dram = nc.dram_tensor("out", [OWN, 128], mybir.dt.float32, kind="ExternalOutput")

    # chunk -> (q, t) and run boundaries, plus first-q per tile
    chunk_meta = []          # (q, t, is_first_of_run, is_last_of_run)
    for q in range(NWIN):
        for t, k in schedule[q]:
            for j in range(k):
                chunk_meta.append((q, t, j == 0, j == k - 1))
    first_q = {}
    for q in range(NWIN):
        for t, _ in schedule[q]:
            first_q.setdefault(t, q)
    last_q = {}
    for q in range(NWIN):
        for t, _ in schedule[q]:
            last_q[t] = q

    with tile.TileContext(nc) as tc:
        with (
            tc.tile_pool(name="persist", bufs=1) as pers,
            tc.tile_pool(name="ybuf", bufs=4) as yp,
            tc.tile_pool(name="mbuf", bufs=4) as mp,
            tc.tile_pool(name="runp", bufs=4, space="PSUM") as rp,
            tc.tile_pool(name="epip", bufs=2, space="PSUM") as ep,
            tc.tile_pool(name="etmp", bufs=4) as et,
            tc.tile_pool(name="xtile", bufs=4) as xp,
            tc.tile_pool(name="dram", bufs=1, space="DRAM") as dp,
        ):
            # ---- persistent SBUF ----
            idx_t = pers.tile([128, CH * 8], mybir.dt.int16, tag="idx")
            nc.sync.dma_start(idx_t[:], idx16_in[:])
            dstl_t = pers.tile([128, CH], mybir.dt.float32, tag="dstl")
            nc.sync.dma_start(dstl_t[:], dstl_in[:])
            coef_t = pers.tile([128, CH], mybir.dt.float32, tag="coef")
            nc.sync.dma_start(coef_t[:], coef_in[:])
            iota_t = pers.tile([128, 128], mybir.dt.float32, tag="iota")
            nc.sync.dma_start(iota_t[:], iota_in[:])
            ident_t = pers.tile([128, 128], mybir.dt.float32, tag="ident")
            nc.sync.dma_start(ident_t[:], ident_in[:])
            w_t, b_t = [], []
            for l in range(4):
                wt = pers.tile([128, 128], mybir.dt.float32, tag=f"w{l}")
                nc.sync.dma_start(wt[:], w_in[l][:])
                w_t.append(wt)
                bt = pers.tile([128, 128], mybir.dt.float32, tag=f"b{l}")
                nc.sync.dma_start(bt[:], b_in[l][:])
                b_t.append(bt)
            agg_t = pers.tile([128, TILES * 128], mybir.dt.float32, tag="agg")
            g_t = pers.tile([128, TILES * 128], mybir.dt.float32, tag="g")

            # ---- collective buffers ----
            cc_in = [dp.tile([OWN, 128], mybir.dt.float32, tag=f"ccin{l}", name=f"ccin{l}")
                     for l in range(4)]
            cc_out = [dp.tile([NT, 128], mybir.dt.float32, tag=f"ccout{l}", name=f"ccout{l}",
                              addr_space="Shared") for l in range(4)]

            def make_xw(l, lhsT_tile, t):
                """psum = lhsT^T @ W_l, write (cast) to cc_in[l] rows of t."""
                pxw = ep.tile([128, 128], mybir.dt.float32, space="PSUM", tag="pxw")
                nc.tensor.matmul(out=pxw[:], lhsT=lhsT_tile[:], rhs=w_t[l][:],
                                 start=True, stop=True)
                xw_sb = et.tile([128, 128], mybir.dt.float32, tag="xwsb")
                nc.scalar.activation(xw_sb[:], pxw[:],
                                     mybir.ActivationFunctionType.Copy)
                nc.sync.dma_start(cc_in[l][t * 128:(t + 1) * 128, :], xw_sb[:])

            # ---- layer 0 pre-phase: xw0 = x @ W0 ----
            for t in range(TILES):
                xt = xp.tile([128, 128], mybir.dt.float32, tag="xt")
                nc.sync.dma_start(xt[:], xT_in[:, t * 128:(t + 1) * 128])
                make_xw(0, xt, t)

            gather_q = [0]

            def do_layer(l):
                nc.gpsimd.collective_compute(
                    "AllGather",
                    mybir.AluOpType.bypass,
                    replica_groups=[list(range(NCORES))],
                    ins=[cc_in[l][:].opt()],
                    outs=[cc_out[l][:].opt()],
                )
                table = cc_out[l]
                # gather calls indexed by chunk ranges
                call_of_chunk = {}
                ybufs = {}
                for (q, c0, c1) in calls:
                    for j in range(c0, c1):
                        call_of_chunk[j] = (q, c0, c1)

                run_psum = None
                for j, (q, t, is_first, is_last) in enumerate(chunk_meta):
                    cq, c0, c1 = call_of_chunk[j]
                    if j == c0:
                        y = yp.tile([128, (c1 - c0), 128], mybir.dt.float32, tag="y")
                        nwin_rows = min(32768, NT - cq * WIN)
                        nc.gpsimd.dma_gather(
                            out_ap=y[:],
                            in_ap=table[cq * WIN:cq * WIN + nwin_rows, :],
                            idxs_ap=idx_t[:, c0 * 8:c1 * 8],
                            num_idxs=(c1 - c0) * 128,
                            num_idxs_reg=(c1 - c0) * 128,
                            elem_size=128,
                            single_packet=False,
                            queue_num=gather_q[0] % 4,
                        )
                        gather_q[0] += 1
                        ybufs[c0] = y
                    y = ybufs[call_of_chunk[j][1]]
                    if is_first:
                        run_psum = rp.tile([128, 128], mybir.dt.float32,
                                           space="PSUM", tag="rp")
                    m = mp.tile([128, 128], mybir.dt.float32, tag="m")
                    nc.vector.tensor_scalar(
                        out=m[:], in0=iota_t[:],
                        scalar1=dstl_t[:, j:j + 1],
                        scalar2=coef_t[:, j:j + 1],
                        op0=mybir.AluOpType.is_equal,
                        op1=mybir.AluOpType.mult,
                    )
                    nc.tensor.matmul(out=run_psum[:], lhsT=m[:],
                                     rhs=y[:, j - c0, :],
                                     start=is_first, stop=is_last)
                    if is_last:
                        agg_sl = agg_t[:, t * 128:(t + 1) * 128]
                        if q == first_q[t]:
                            nc.vector.tensor_copy(agg_sl, run_psum[:])
                        else:
                            nc.vector.tensor_tensor(
                                out=agg_sl, in0=agg_sl, in1=run_psum[:],
                                op=mybir.AluOpType.add)
                        if q == last_q[t]:
                            epilogue(l, t)

            def epilogue(l, t):
                agg_sl = agg_t[:, t * 128:(t + 1) * 128]
                g_sl = g_t[:, t * 128:(t + 1) * 128]
                h = et.tile([128, 128], mybir.dt.float32, tag="h")
                nc.vector.tensor_tensor(out=h[:], in0=agg_sl, in1=b_t[l][:],
                                        op=mybir.AluOpType.add)
                if l in (1, 2):
                    nc.vector.tensor_tensor(out=h[:], in0=h[:], in1=g_sl,
                                            op=mybir.AluOpType.add)
                if l == 3:
                    nc.sync.dma_start(out_dram[t * 128:(t + 1) * 128, :], h[:])
                    return
                nc.scalar.activation(g_sl, h[:],
                                     mybir.ActivationFunctionType.Gelu)
                pgt = ep.tile([128, 128], mybir.dt.float32, space="PSUM",
                              tag="pgt")
                nc.tensor.transpose(out=pgt[:], in_=g_sl, identity=ident_t[:])
                gt_sb = et.tile([128, 128], mybir.dt.float32, tag="gt")
                nc.scalar.activation(gt_sb[:], pgt[:],
                                     mybir.ActivationFunctionType.Copy)
                make_xw(l + 1, gt_sb, t)

            for l in range(4):
                do_layer(l)

    nc.compile()
    return nc


# --------------------------------------------------------------------------
# public entry point
# --------------------------------------------------------------------------

def _host_inputs(x, edge_index, Ws, bs):
    n_real = x.shape[0]
    per_core, schedule, CH = preprocess(x, edge_index, n_real)
    calls = _call_plan(schedule)
    call_slices = [(c0, c1) for (_, c0, c1) in calls]

    xpad = np.zeros((NT, F), dtype=np.float32)
    xpad[:n_real] = np.asarray(x, dtype=np.float32)

    W3p = np.zeros((128, 128), np.float32)
    W3p[:, :C_OUT] = Ws[3]
    Wl = [np.asarray(Ws[0], np.float32), np.asarray(Ws[1], np.float32),
          np.asarray(Ws[2], np.float32), W3p]
    b3p = np.zeros(128, np.float32)
    b3p[:C_OUT] = bs[3]
    bl = [np.asarray(bs[0], np.float32), np.asarray(bs[1], np.float32),
          np.asarray(bs[2], np.float32), b3p]

    iota = np.tile(np.arange(128, dtype=np.float32), (128, 1))
    ident = np.eye(128, dtype=np.float32)

    in_maps = []
    for c in range(NCORES):
        d = per_core[c]
        m = {
            "xT": xpad[c * OWN:(c + 1) * OWN].T.copy(),
            "idx16": _wrap_idx(d["idx16"], call_slices),
            "dstl": d["dstl"].reshape(-1, 128).T.copy(),
            "coef": d["coef"].reshape(-1, 128).T.copy(),
            "iota": iota, "ident": ident,
        }
        for l in range(4):
            m[f"W{l}"] = Wl[l]
            m[f"b{l}"] = np.tile(bl[l], (128, 1))
        in_maps.append(m)
    return in_maps, schedule, CH


def kernel(x, edge_index, W0, b0, W1, b1, W2, b2, W3, b3):
    x = np.asarray(x)
    in_maps, schedule, CH = _host_inputs(
        x, np.asarray(edge_index), [W0, W1, W2, W3], [b0, b1, b2, b3])
    nc = build(schedule, CH)
    res = run_bass_kernel_spmd(nc, in_maps, list(range(NCORES)))
    outs = [res.results[c]["out"] for c in range(NCORES)]
    full = np.concatenate(outs, axis=0)[:x.shape[0], :C_OUT]
    return full.astype(np.float32)
